# revision 12
# baseline (speedup 1.0000x reference)
"""Trainium2 Bass kernel for a dense pre-LN transformer block.

Reference computation (B=2, T=2048, E=1024, H=16, HS=64):
    h  = LN(x; g1, be1)
    q, k, v = per-head projections of h        (wq/wk/wv: [H, E, HS])
    att = causal softmax(q k^T / sqrt(E)) v    (per head)
    x2 = x + concat(att) @ w_proj + b_proj
    y  = x2 + relu(LN(x2; g2, be2) @ w1 + b1) @ w2 + b2

Distribution over 8 NeuronCores:
  - QKV + attention: tensor-parallel over heads (2 heads/core).
  - proj + FFN: data-parallel over token rows (512 tokens/core).
  - One AllToAll (attn output, feature-major) glues the two.

Device-side layout is feature-major ("transposed"): activations are
[feature, token]; the host pre-transposes x and pre-arranges weights so
the kernel never transposes activations on device.

LayerNorm is folded into the following matmul:
    LN(x) @ W = r_t * (x @ (g*W) - mu_t * colsum(g*W) + sigma_t * (be@W))
applied via two rank-1 "augmentation" rows in the contraction, where
mu/sigma/r are per-token stats computed with ones-matmuls (bf16, exact
to ~1e-4). Softmax runs without max-subtraction (logits are bounded by
design: scale 1/sqrt(E)=1/32); the per-token r for V is folded into the
exp() bias as ln(r_s), and the softmax denominator is obtained from a
sigma-column appended to V in the PV matmul.
"""

import sys
from contextlib import ExitStack
import numpy as np

sys.path.insert(0, "/opt/trn_rl_repo")

# ---------------------------------------------------------------- constants
B, T, E, H = 2, 2048, 1024, 16
HS = E // H          # 64
W = 8                # cores
BT = B * T           # 4096 tokens total
TB = BT // W         # 512 tokens per core (row-DP block)
HPC = H // W         # 2 heads per core
FCH = E // 128       # 8 feature chunks
M4E = 4 * E          # 4096 ffn hidden
MCH = M4E // 128     # 32 hidden chunks
TT = 512             # token tile (matmul moving dim)
NTT = BT // TT       # 8 token tiles
SCH = BT // 128      # 32 token chunks of 128 (for v / s-chunks)
EPS = 1e-5

# dtype knobs: "f32" or "bf16"
import os as _os
MM_DT_S = _os.environ.get("KMM_DT", "bf16")    # linear-layer matmuls
ATT_DT_S = _os.environ.get("KATT_DT", "bf16")  # attention matmuls

_CACHE = {}


def _build(nc, mm_dt, att_dt, f32):
    import concourse.bass as bass
    from concourse.tile import TileContext
    import concourse.mybir as mybir

    AF = mybir.ActivationFunctionType
    dp = nc.declare_dram_parameter

    use_f32_x = mm_dt == f32  # stream fp32 x for linear matmuls

    # ------------------------------------------------- DRAM parameters
    if use_f32_x:
        xT_d = dp("xT", [FCH, 128, BT], f32, isOutput=False)
    xh_d = dp("xh", [FCH, 128, BT], mybir.dt.bfloat16, isOutput=False)
    xtb_d = dp("xtb", [FCH, 128, TB], f32, isOutput=False)
    wqkv_d = dp("wqkv", [FCH, 128, 3 * 128], mm_dt, isOutput=False)
    augw_d = dp("augw", [2, 3 * 128], mm_dt, isOutput=False)
    wproj_d = dp("wproj", [FCH, 128, E], mm_dt, isOutput=False)
    bproj_d = dp("bproj", [128, FCH], f32, isOutput=False)
    w1_d = dp("w1", [FCH, 128, M4E], mm_dt, isOutput=False)
    augw1_d = dp("augw1", [2, M4E], mm_dt, isOutput=False)
    w2_d = dp("w2", [FCH, 128, M4E], mm_dt, isOutput=False)  # [et,p,(mc,f)] host layout
    b2_d = dp("b2c", [128, FCH], f32, isOutput=False)
    mask_d = dp("mask", [4, 128, TT], att_dt, isOutput=False)
    y_d = dp("y", [FCH, 128, TB], f32, isOutput=True)


    es = ExitStack()
    with TileContext(nc) as tc, es:
        # ------------------------------------------------- pools
        glob = es.enter_context(tc.tile_pool(name="glob", bufs=1))
        dramp = es.enter_context(tc.tile_pool(name="dramp", bufs=1, space="DRAM"))
        a2a_in = dramp.tile([W, 128, TB], mm_dt, tag="a2a_in")
        a2a_out = dramp.tile([W, 128, TB], mm_dt, tag="a2a_out")
        psb = es.enter_context(tc.tile_pool(name="psb", bufs=6, space="PSUM"))
        pss = es.enter_context(tc.tile_pool(name="pss", bufs=2, space="PSUM"))

        ones_bf = glob.tile([128, 1], mybir.dt.bfloat16, tag="ones_bf")
        nc.vector.memset(ones_bf[:], 1.0)
        ones_f1 = glob.tile([1, 1], f32, tag="ones_f1")
        nc.vector.memset(ones_f1[:], 1.0)

        eps1 = glob.tile([1, 1], f32, tag="eps1")
        nc.vector.memset(eps1[:], EPS)
        ln_r_cols = glob.tile([128, SCH], f32, tag="ln_r_cols")
        sig_cols = glob.tile([128, SCH], f32, tag="sig_cols")
        attn_slab = glob.tile([128, BT], mm_dt, tag="attn_slab")

        # ========================================================= PHASE A
        es_a = es.enter_context(ExitStack())
        pha = es_a.enter_context(tc.tile_pool(name="pha", bufs=1))
        str_a = es_a.enter_context(tc.tile_pool(name="stra", bufs=2))
        expp = es_a.enter_context(tc.tile_pool(name="expp", bufs=4))

        wqkv = pha.tile([128, FCH, 3 * 128], mm_dt, tag="wqkv")
        for o in range(FCH):
            nc.sync.dma_start(wqkv[:, o, :], wqkv_d[o])
        augw = pha.tile([2, 3 * 128], mm_dt, tag="augw")
        nc.sync.dma_start(augw[:], augw_d[:])
        masks = pha.tile([128, 4, TT], att_dt, tag="masks")
        for kk in range(4):
            nc.sync.dma_start(masks[:, kk, :], mask_d[kk])

        sig_rows = pha.tile([1, BT], f32, tag="sig_rows")
        r_rows = pha.tile([1, BT], f32, tag="r_rows")
        qT = pha.tile([128, BT], att_dt, tag="qT")
        kT = pha.tile([128, BT], att_dt, tag="kT")
        v_slab = pha.tile([128, SCH, 130], att_dt, tag="v_slab")

        # ---- per token-tile: stats + QKV
        for tj in range(NTT):
            tsl = slice(tj * TT, (tj + 1) * TT)
            if use_f32_x:
                xt = str_a.tile([128, FCH, TT], f32, tag="xt")
                for o in range(FCH):
                    nc.sync.dma_start(xt[:, o, :], xT_d[o, :, tsl])
            xh = str_a.tile([128, FCH, TT], mybir.dt.bfloat16, tag="xh")
            for o in range(FCH):
                nc.sync.dma_start(xh[:, o, :], xh_d[o, :, tsl])

            # token stats: sum(x), sum(x^2) over features (bf16 ones-matmuls)
            ps_sum = pss.tile([1, TT], f32, tag="small")
            ps_sq = pss.tile([1, TT], f32, tag="small")
            for o in range(FCH):
                sq = str_a.tile([128, TT], mybir.dt.bfloat16, tag="sq")
                nc.scalar.activation(sq[:], xh[:, o, :], AF.Square)
                nc.tensor.matmul(ps_sum[:], ones_bf[:], xh[:, o, :],
                                 start=(o == 0), stop=(o == FCH - 1))
                nc.tensor.matmul(ps_sq[:], ones_bf[:], sq[:],
                                 start=(o == 0), stop=(o == FCH - 1))

            mu_row = str_a.tile([1, TT], f32, tag="mu_row")
            nc.scalar.activation(mu_row[:], ps_sum[:], AF.Copy, scale=1.0 / E)
            msq_row = str_a.tile([1, TT], f32, tag="msq_row")
            nc.scalar.activation(msq_row[:], ps_sq[:], AF.Copy, scale=1.0 / E)
            var_row = str_a.tile([1, TT], f32, tag="var_row")
            nc.vector.tensor_mul(var_row[:], mu_row[:], mu_row[:])
            nc.vector.tensor_sub(var_row[:], msq_row[:], var_row[:])
            sig_row = sig_rows[0:1, tsl]
            nc.scalar.activation(sig_row, var_row[:], AF.Sqrt, bias=eps1[:])
            r_row = r_rows[0:1, tsl]
            nc.vector.reciprocal(r_row, sig_row)

            aug_rhs = str_a.tile([2, TT], mm_dt, tag="aug_rhs")
            nc.vector.tensor_copy(aug_rhs[0:1, :], mu_row[:])
            nc.gpsimd.dma_start(aug_rhs[1:2, :], sig_row)

            r_b = str_a.tile([128, TT], f32, tag="r_b")
            nc.gpsimd.partition_broadcast(r_b[:], r_row)

            x_mm = xt if use_f32_x else xh

            # q, k: out = [feat, tok]
            for fg, slab in ((0, qT), (1, kT)):
                fsl = slice(fg * 128, (fg + 1) * 128)
                ps = psb.tile([128, TT], f32, tag="big")
                for o in range(FCH):
                    nc.tensor.matmul(ps[:], wqkv[:, o, fsl], x_mm[:, o, :],
                                     start=(o == 0), stop=False)
                nc.tensor.matmul(ps[:], augw[:, fsl], aug_rhs[:],
                                 start=False, stop=True)
                nc.vector.tensor_mul(slab[:, tsl], ps[:], r_b[:])

            # v: out = [tok, feat] (row-major, for PV lhsT)
            for j in range(4):
                g = tj * 4 + j
                csl = slice(j * 128, (j + 1) * 128)
                ps = psb.tile([128, 128], f32, tag="big")
                for o in range(FCH):
                    nc.tensor.matmul(ps[:], x_mm[:, o, csl],
                                     wqkv[:, o, 256:384],
                                     start=(o == 0), stop=False)
                nc.tensor.matmul(ps[:], aug_rhs[:, csl], augw[:, 256:384],
                                 start=False, stop=True)
                nc.scalar.activation(v_slab[:, g, 0:64], ps[:, 0:64], AF.Copy)
                nc.scalar.activation(v_slab[:, g, 65:129], ps[:, 64:128], AF.Copy)

        # ---- dense block: r/sigma as columns for all 32 token chunks
        for g in range(SCH):
            csl = slice(g * 128, (g + 1) * 128)
            pr = pss.tile([128, 1], f32, tag="small")
            nc.tensor.matmul(pr[:], r_rows[0:1, csl], ones_f1[:])
            nc.scalar.activation(ln_r_cols[:, g:g + 1], pr[:], AF.Ln)
            psg = pss.tile([128, 1], f32, tag="small")
            nc.tensor.matmul(psg[:], sig_rows[0:1, csl], ones_f1[:])
            nc.scalar.activation(sig_cols[:, g:g + 1], psg[:], AF.Copy)
            nc.vector.tensor_copy(v_slab[:, g, 64:65], sig_cols[:, g:g + 1])
            nc.vector.tensor_copy(v_slab[:, g, 129:130], sig_cols[:, g:g + 1])

        # ---- attention (2 local heads, causal, no-max softmax)
        for b in range(B):
            for tj in range(T // TT):
                tsl = slice(b * T + tj * TT, b * T + (tj + 1) * TT)
                po0 = psb.tile([65, TT], f32, tag="big")
                po1 = psb.tile([65, TT], f32, tag="big")
                n_si = 4 * tj + 4
                for si in range(n_si):
                    g = b * (T // 128) + si
                    ssl = slice(g * 128, (g + 1) * 128)
                    pss_ = []
                    for hh in (0, 1):
                        hsl = slice(hh * 64, (hh + 1) * 64)
                        ps = psb.tile([128, TT], f32, tag="big")
                        nc.tensor.matmul(ps[:], kT[hsl, ssl], qT[hsl, tsl],
                                         start=True, stop=True)
                        pss_.append(ps)
                    exs = []
                    for hh, po in ((0, po0), (1, po1)):
                        ex = expp.tile([128, TT], att_dt, tag="ex")
                        nc.scalar.activation(
                            ex[:], pss_[hh][:], AF.Exp,
                            scale=float(E) ** -0.5,
                            bias=ln_r_cols[:, g:g + 1],
                        )
                        if si >= 4 * tj:
                            nc.vector.tensor_mul(ex[:], ex[:],
                                                 masks[:, si - 4 * tj, :])
                        exs.append(ex)
                    for hh, po in ((0, po0), (1, po1)):
                        nc.tensor.matmul(po[:], v_slab[:, g, 65 * hh:65 * hh + 65],
                                         exs[hh][:], start=(si == 0),
                                         stop=(si == n_si - 1))
                for hh, po in ((0, po0), (1, po1)):
                    rd = expp.tile([1, TT], f32, tag="rd")
                    nc.vector.reciprocal(rd[:], po[64:65, :])
                    rb = expp.tile([64, TT], f32, tag="rb")
                    nc.gpsimd.partition_broadcast(rb[:], rd[:])
                    nc.vector.tensor_mul(attn_slab[hh * 64:(hh + 1) * 64, tsl],
                                         po[0:64, :], rb[:])
                d = b * (T // TT) + tj
                nc.sync.dma_start(a2a_in[d], attn_slab[:, d * TB:(d + 1) * TB])

        es_a.close()

        # ========================================================= A2A
        nc.gpsimd.collective_compute(
            "AllToAll", mybir.AluOpType.bypass,
            ins=[a2a_in.opt()], outs=[a2a_out.opt()],
            replica_groups=[list(range(W))],
        )

        # ========================================================= PHASE B
        phb = es.enter_context(tc.tile_pool(name="phb", bufs=1))
        str_b = es.enter_context(tc.tile_pool(name="strb", bufs=2))

        atf = phb.tile([128, FCH, TB], mm_dt, tag="atf")
        for o in range(FCH):
            nc.sync.dma_start(atf[:, o, :], a2a_out[o])
        xtb = phb.tile([128, FCH, TB], f32, tag="xtb")
        for o in range(FCH):
            nc.sync.dma_start(xtb[:, o, :], xtb_d[o])
        bproj = phb.tile([128, FCH], f32, tag="bproj")
        nc.sync.dma_start(bproj[:], bproj_d[:])
        b2c = phb.tile([128, FCH], f32, tag="b2c")
        nc.sync.dma_start(b2c[:], b2_d[:])
        augw1 = phb.tile([2, M4E], mm_dt, tag="augw1")
        nc.sync.dma_start(augw1[:], augw1_d[:])

        # ---- proj + residual -> x2T
        x2T = phb.tile([128, FCH, TB], f32, tag="x2T")
        for et in range(FCH):
            esl = slice(et * 128, (et + 1) * 128)
            wp = str_b.tile([128, FCH, 128], mm_dt, tag="wp")
            for o in range(FCH):
                nc.sync.dma_start(wp[:, o, :], wproj_d[o, :, esl])
            ps = psb.tile([128, TB], f32, tag="big")
            for o in range(FCH):
                nc.tensor.matmul(ps[:], wp[:, o, :], atf[:, o, :],
                                 start=(o == 0), stop=(o == FCH - 1))
            nc.vector.scalar_tensor_tensor(
                x2T[:, et, :], ps[:], bproj[:, et:et + 1], xtb[:, et, :],
                mybir.AluOpType.add, mybir.AluOpType.add)

        # ---- LN2 stats
        xh2 = phb.tile([128, FCH, TB], mybir.dt.bfloat16, tag="xh2")
        ps_sum = pss.tile([1, TB], f32, tag="small")
        ps_sq = pss.tile([1, TB], f32, tag="small")
        for o in range(FCH):
            nc.vector.tensor_copy(xh2[:, o, :], x2T[:, o, :])
            sq = str_b.tile([128, TB], mybir.dt.bfloat16, tag="sq2")
            nc.scalar.activation(sq[:], xh2[:, o, :], AF.Square)
            nc.tensor.matmul(ps_sum[:], ones_bf[:], xh2[:, o, :],
                             start=(o == 0), stop=(o == FCH - 1))
            nc.tensor.matmul(ps_sq[:], ones_bf[:], sq[:],
                             start=(o == 0), stop=(o == FCH - 1))
        mu2 = phb.tile([1, TB], f32, tag="mu2")
        nc.scalar.activation(mu2[:], ps_sum[:], AF.Copy, scale=1.0 / E)
        msq2 = phb.tile([1, TB], f32, tag="msq2")
        nc.scalar.activation(msq2[:], ps_sq[:], AF.Copy, scale=1.0 / E)
        var2 = phb.tile([1, TB], f32, tag="var2")
        nc.vector.tensor_mul(var2[:], mu2[:], mu2[:])
        nc.vector.tensor_sub(var2[:], msq2[:], var2[:])
        sig2 = phb.tile([1, TB], f32, tag="sig2")
        nc.scalar.activation(sig2[:], var2[:], AF.Sqrt, bias=eps1[:])
        r2 = phb.tile([1, TB], f32, tag="r2")
        nc.vector.reciprocal(r2[:], sig2[:])
        aug2 = phb.tile([2, TB], mm_dt, tag="aug2")
        nc.vector.tensor_copy(aug2[0:1, :], mu2[:])
        nc.gpsimd.dma_start(aug2[1:2, :], sig2[:])
        r2_b = phb.tile([128, TB], f32, tag="r2_b")
        nc.gpsimd.partition_broadcast(r2_b[:], r2[:])

        x2mm = x2T if use_f32_x else xh2

        # ---- FFN1 -> relu slab (r2 deferred to FFN2 output: r2>0)
        relu = phb.tile([128, MCH, TB], mm_dt, tag="relu")
        for mt in range(MCH):
            msl = slice(mt * 128, (mt + 1) * 128)
            w1t = str_b.tile([128, FCH, 128], mm_dt, tag="w1t")
            for o in range(FCH):
                nc.sync.dma_start(w1t[:, o, :], w1_d[o, :, msl])
            ps = psb.tile([128, TB], f32, tag="big")
            for o in range(FCH):
                nc.tensor.matmul(ps[:], w1t[:, o, :], x2mm[:, o, :],
                                 start=(o == 0), stop=False)
            nc.tensor.matmul(ps[:], augw1[:, msl], aug2[:],
                             start=False, stop=True)
            nc.scalar.activation(relu[:, mt, :], ps[:], AF.Relu)

        # ---- FFN2 + r2 + residual + b2 -> y
        for et in range(FCH):
            ps = psb.tile([128, TB], f32, tag="big")
            for qq in range(4):
                w2t = str_b.tile([128, MCH // 4, 128], mm_dt, tag="w2t")
                nc.sync.dma_start(
                    w2t[:],
                    w2_d[et, :, qq * (M4E // 4):(qq + 1) * (M4E // 4)]
                    .rearrange("p (m f) -> p m f", f=128))
                for mj in range(MCH // 4):
                    mc = qq * (MCH // 4) + mj
                    nc.tensor.matmul(ps[:], w2t[:, mj, :], relu[:, mc, :],
                                     start=(mc == 0), stop=(mc == MCH - 1))
            u = str_b.tile([128, TB], f32, tag="u")
            nc.vector.tensor_mul(u[:], ps[:], r2_b[:])
            yt = str_b.tile([128, TB], f32, tag="yt")
            nc.vector.scalar_tensor_tensor(
                yt[:], u[:], b2c[:, et:et + 1], x2T[:, et, :],
                mybir.AluOpType.add, mybir.AluOpType.add)
            nc.sync.dma_start(y_d[et], yt[:])


    nc.finalize()
    return nc


def _get_nc():
    key = (MM_DT_S, ATT_DT_S)
    if key in _CACHE:
        return _CACHE[key]
    from concourse import bacc
    import concourse.mybir as mybir

    f32 = mybir.dt.float32
    mm_dt = f32 if MM_DT_S == "f32" else mybir.dt.bfloat16
    att_dt = f32 if ATT_DT_S == "f32" else mybir.dt.bfloat16
    nc = bacc.Bacc("TRN2", target_bir_lowering=False, debug=False,
                   num_devices=W)
    _build(nc, mm_dt, att_dt, f32)
    _CACHE[key] = nc
    return nc


def _prep_inputs(x, wq, wk, wv, w_proj, b_proj, w1, b1, w2, b2, g1, be1, g2, be2):
    """Host-side sharding: returns in_maps (list of 8 dicts)."""
    import ml_dtypes

    bf16 = ml_dtypes.bfloat16
    mm_np = np.float32 if MM_DT_S == "f32" else bf16
    att_np = np.float32 if ATT_DT_S == "f32" else bf16

    xf = np.ascontiguousarray(x.reshape(BT, E).T)          # [E, BT]
    xT = xf.reshape(FCH, 128, BT)
    xh = xT.astype(bf16)

    # causal mask tiles for the 4 diagonal-crossing offsets
    mask = np.zeros((4, 128, TT), dtype=att_np)
    uu = np.arange(TT)[None, :]
    pp = np.arange(128)[:, None]
    for k in range(4):
        mask[k] = (pp <= uu - 128 * k).astype(att_np)

    wproj_l = np.ascontiguousarray(w_proj.reshape(FCH, 128, E)).astype(mm_np)
    bproj_l = np.ascontiguousarray(b_proj.reshape(FCH, 128).T)  # [128, FCH]

    w1s = (g2[:, None] * w1)                                # [E, 4E]
    w1_l = np.ascontiguousarray(w1s.reshape(FCH, 128, M4E)).astype(mm_np)
    aug1 = np.stack([-w1s.sum(axis=0), be2 @ w1 + b1]).astype(mm_np)

    # w2 host layout: [et, p, (mc, f)] with w2_l[et, p, mc*128+f] = w2[mc*128+p, et*128+f]
    w2r = w2.reshape(MCH, 128, FCH, 128)                    # [mc, p, et, f]
    w2_l = np.ascontiguousarray(w2r.transpose(2, 1, 0, 3).reshape(FCH, 128, M4E)).astype(mm_np)
    b2_l = np.ascontiguousarray(b2.reshape(FCH, 128).T)

    in_maps = []
    for c in range(W):
        hsl = slice(HPC * c, HPC * (c + 1))
        wq_c = wq[hsl].transpose(1, 0, 2).reshape(E, 128)
        wk_c = wk[hsl].transpose(1, 0, 2).reshape(E, 128)
        wv_c = wv[hsl].transpose(1, 0, 2).reshape(E, 128)
        wqkv = np.concatenate([g1[:, None] * wq_c,
                               g1[:, None] * wk_c,
                               g1[:, None] * wv_c], axis=1)  # [E, 384]
        augw = np.stack([-wqkv.sum(axis=0),
                         np.concatenate([be1 @ wq_c, be1 @ wk_c, be1 @ wv_c])]
                        ).astype(mm_np)
        m = {
            "xh": xh,
            "xtb": np.ascontiguousarray(xT[:, :, TB * c:TB * (c + 1)]),
            "wqkv": np.ascontiguousarray(wqkv.reshape(FCH, 128, 384)).astype(mm_np),
            "augw": augw,
            "wproj": wproj_l,
            "bproj": np.ascontiguousarray(bproj_l),
            "w1": w1_l,
            "augw1": aug1,
            "w2": w2_l,
            "b2c": np.ascontiguousarray(b2_l),
            "mask": mask,
        }
        if MM_DT_S == "f32":
            m["xT"] = xT
        in_maps.append(m)
    return in_maps


def kernel(**inputs):
    from concourse.bass_utils import run_bass_kernel_spmd

    nc = _get_nc()
    in_maps = _prep_inputs(**{k: np.asarray(v) for k, v in inputs.items()})
    res = run_bass_kernel_spmd(nc, in_maps, list(range(W)))
    # gather: core c produced y = [FCH, 128, TB] = yT block for tokens [TB*c, TB*(c+1))
    out_T = np.concatenate([res.results[c]["y"].reshape(E, TB)
                            for c in range(W)], axis=1)      # [E, BT]
    return np.ascontiguousarray(out_T.T).reshape(B, T, E).astype(np.float32)


# revision 15
# speedup vs baseline: 1.1951x; 1.1951x over previous
"""Trainium2 Bass kernel for a dense pre-LN transformer block.

Reference computation (B=2, T=2048, E=1024, H=16, HS=64):
    h  = LN(x; g1, be1)
    q, k, v = per-head projections of h        (wq/wk/wv: [H, E, HS])
    att = causal softmax(q k^T / sqrt(E)) v    (per head)
    x2 = x + concat(att) @ w_proj + b_proj
    y  = x2 + relu(LN(x2; g2, be2) @ w1 + b1) @ w2 + b2

Distribution over 8 NeuronCores:
  - QKV + attention: tensor-parallel over heads (2 heads/core).
  - proj + FFN: data-parallel over token rows (512 tokens/core).
  - One AllToAll (attn output, feature-major) glues the two.

Device-side layout is feature-major ("transposed"): activations are
[feature, token]; the host pre-transposes x and pre-arranges weights so
the kernel never transposes activations on device.

LayerNorm is folded into the following matmul:
    LN(x) @ W = r_t * (x @ (g*W) - mu_t * colsum(g*W) + sigma_t * (be@W))
applied via two rank-1 "augmentation" rows in the contraction, where
mu/sigma/r are per-token stats computed with ones-matmuls (bf16, exact
to ~1e-4). Softmax runs without max-subtraction (logits are bounded by
design: scale 1/sqrt(E)=1/32); the per-token r for V is folded into the
exp() bias as ln(r_s), and the softmax denominator is obtained from a
sigma-column appended to V in the PV matmul.
"""

import sys
from contextlib import ExitStack
import numpy as np

sys.path.insert(0, "/opt/trn_rl_repo")

# ---------------------------------------------------------------- constants
B, T, E, H = 2, 2048, 1024, 16
HS = E // H          # 64
W = 8                # cores
BT = B * T           # 4096 tokens total
TB = BT // W         # 512 tokens per core (row-DP block)
HPC = H // W         # 2 heads per core
FCH = E // 128       # 8 feature chunks
M4E = 4 * E          # 4096 ffn hidden
MCH = M4E // 128     # 32 hidden chunks
TT = 512             # token tile (matmul moving dim)
NTT = BT // TT       # 8 token tiles
SCH = BT // 128      # 32 token chunks of 128 (for v / s-chunks)
EPS = 1e-5

# dtype knobs: "f32" or "bf16"
import os as _os
MM_DT_S = _os.environ.get("KMM_DT", "bf16")    # linear-layer matmuls
ATT_DT_S = _os.environ.get("KATT_DT", "bf16")  # attention matmuls

_CACHE = {}


def _build(nc, mm_dt, att_dt, f32):
    import concourse.bass as bass
    from concourse.tile import TileContext
    import concourse.mybir as mybir

    AF = mybir.ActivationFunctionType
    dp = nc.declare_dram_parameter

    use_f32_x = mm_dt == f32  # stream fp32 x for linear matmuls

    # ------------------------------------------------- DRAM parameters
    if use_f32_x:
        xT_d = dp("xT", [FCH, 128, BT], f32, isOutput=False)
    xh_d = dp("xh", [FCH, 128, BT], mybir.dt.bfloat16, isOutput=False)
    xtb_d = dp("xtb", [FCH, 128, TB], f32, isOutput=False)
    wqkv_d = dp("wqkv", [FCH, 128, 3 * 128], mm_dt, isOutput=False)
    augw_d = dp("augw", [2, 3 * 128], mm_dt, isOutput=False)
    wproj_d = dp("wproj", [FCH, 128, E], mm_dt, isOutput=False)
    bproj_d = dp("bproj", [128, FCH], f32, isOutput=False)
    w1_d = dp("w1", [FCH, 128, M4E], mm_dt, isOutput=False)  # [s][p][(o,c512)] layout
    augw1_d = dp("augw1", [2, M4E], mm_dt, isOutput=False)
    w2_d = dp("w2", [FCH, 128, M4E], mm_dt, isOutput=False)  # [et,p,(mc,f)] host layout
    b2_d = dp("b2c", [128, FCH], f32, isOutput=False)
    mask_d = dp("mask", [4, 128, TT], att_dt, isOutput=False)
    y_d = dp("y", [FCH, 128, TB], f32, isOutput=True)


    es = ExitStack()
    with TileContext(nc) as tc, es:
        # ------------------------------------------------- pools
        glob = es.enter_context(tc.tile_pool(name="glob", bufs=1))
        dramp = es.enter_context(tc.tile_pool(name="dramp", bufs=1, space="DRAM"))
        a2a_in = [dramp.tile([W, 64, TB], mm_dt, tag=f"a2a_in{h}", name=f"a2a_in{h}") for h in (0, 1)]
        a2a_out = [dramp.tile([W, 64, TB], mm_dt, tag=f"a2a_out{h}", name=f"a2a_out{h}") for h in (0, 1)]
        psb = es.enter_context(tc.tile_pool(name="psb", bufs=6, space="PSUM"))
        pss = es.enter_context(tc.tile_pool(name="pss", bufs=2, space="PSUM"))

        ones_bf = glob.tile([128, 1], mybir.dt.bfloat16, tag="ones_bf")
        nc.vector.memset(ones_bf[:], 1.0)
        ones_f1 = glob.tile([1, 1], f32, tag="ones_f1")
        nc.vector.memset(ones_f1[:], 1.0)

        eps1 = glob.tile([1, 1], f32, tag="eps1")
        nc.vector.memset(eps1[:], EPS)
        ln_r_cols = glob.tile([128, SCH], f32, tag="ln_r_cols")
        sig_cols = glob.tile([128, SCH], f32, tag="sig_cols")
        attn_slab = glob.tile([128, BT], mm_dt, tag="attn_slab")

        # ========================================================= PHASE A
        es_a = es.enter_context(ExitStack())
        pha = es_a.enter_context(tc.tile_pool(name="pha", bufs=1))
        str_a = es_a.enter_context(tc.tile_pool(name="stra", bufs=2))
        expp = es_a.enter_context(tc.tile_pool(name="expp", bufs=6))

        wqkv = pha.tile([128, FCH, 3 * 128], mm_dt, tag="wqkv")
        for o in range(FCH):
            nc.sync.dma_start(wqkv[:, o, :], wqkv_d[o])
        augw = pha.tile([2, 3 * 128], mm_dt, tag="augw")
        nc.sync.dma_start(augw[:], augw_d[:])
        masks = pha.tile([128, 4, TT], att_dt, tag="masks")
        for kk in range(4):
            nc.sync.dma_start(masks[:, kk, :], mask_d[kk])

        sig_rows = pha.tile([1, BT], f32, tag="sig_rows")
        r_rows = pha.tile([1, BT], f32, tag="r_rows")
        qT = pha.tile([128, BT], att_dt, tag="qT")
        kT = pha.tile([128, BT], att_dt, tag="kT")
        v_slab = pha.tile([128, SCH, 130], att_dt, tag="v_slab")

        # ---- per token-tile: stats + QKV
        for tj in range(NTT):
            tsl = slice(tj * TT, (tj + 1) * TT)
            if use_f32_x:
                xt = str_a.tile([128, FCH, TT], f32, tag="xt")
                for o in range(FCH):
                    nc.sync.dma_start(xt[:, o, :], xT_d[o, :, tsl])
            xh = str_a.tile([128, FCH, TT], mybir.dt.bfloat16, tag="xh")
            for o in range(FCH):
                nc.sync.dma_start(xh[:, o, :], xh_d[o, :, tsl])

            # token stats: sum(x), sum(x^2) over features (bf16 ones-matmuls)
            ps_sum = pss.tile([1, TT], f32, tag="small")
            ps_sq = pss.tile([1, TT], f32, tag="small")
            for o in range(FCH):
                sq = str_a.tile([128, TT], mybir.dt.bfloat16, tag="sq")
                nc.scalar.activation(sq[:], xh[:, o, :], AF.Square)
                nc.tensor.matmul(ps_sum[:], ones_bf[:], xh[:, o, :],
                                 start=(o == 0), stop=(o == FCH - 1))
                nc.tensor.matmul(ps_sq[:], ones_bf[:], sq[:],
                                 start=(o == 0), stop=(o == FCH - 1))

            mu_row = str_a.tile([1, TT], f32, tag="mu_row")
            nc.scalar.activation(mu_row[:], ps_sum[:], AF.Copy, scale=1.0 / E)
            msq_row = str_a.tile([1, TT], f32, tag="msq_row")
            nc.scalar.activation(msq_row[:], ps_sq[:], AF.Copy, scale=1.0 / E)
            var_row = str_a.tile([1, TT], f32, tag="var_row")
            nc.vector.tensor_mul(var_row[:], mu_row[:], mu_row[:])
            nc.vector.tensor_sub(var_row[:], msq_row[:], var_row[:])
            sig_row = sig_rows[0:1, tsl]
            nc.scalar.activation(sig_row, var_row[:], AF.Sqrt, bias=eps1[:])
            r_row = r_rows[0:1, tsl]
            nc.vector.reciprocal(r_row, sig_row)

            aug_rhs = str_a.tile([2, TT], mm_dt, tag="aug_rhs")
            nc.vector.tensor_copy(aug_rhs[0:1, :], mu_row[:])
            nc.gpsimd.dma_start(aug_rhs[1:2, :], sig_row)

            r_b = str_a.tile([128, TT], f32, tag="r_b")
            nc.gpsimd.partition_broadcast(r_b[:], r_row)

            x_mm = xt if use_f32_x else xh

            # q, k: out = [feat, tok]
            for fg, slab in ((0, qT), (1, kT)):
                fsl = slice(fg * 128, (fg + 1) * 128)
                ps = psb.tile([128, TT], f32, tag="big")
                for o in range(FCH):
                    nc.tensor.matmul(ps[:], wqkv[:, o, fsl], x_mm[:, o, :],
                                     start=(o == 0), stop=False)
                nc.tensor.matmul(ps[:], augw[:, fsl], aug_rhs[:],
                                 start=False, stop=True)
                nc.vector.tensor_mul(slab[:, tsl], ps[:], r_b[:])

            # v: out = [tok, feat] (row-major, for PV lhsT)
            for j in range(4):
                g = tj * 4 + j
                csl = slice(j * 128, (j + 1) * 128)
                ps = psb.tile([128, 128], f32, tag="big")
                for o in range(FCH):
                    nc.tensor.matmul(ps[:], x_mm[:, o, csl],
                                     wqkv[:, o, 256:384],
                                     start=(o == 0), stop=False)
                nc.tensor.matmul(ps[:], aug_rhs[:, csl], augw[:, 256:384],
                                 start=False, stop=True)
                nc.scalar.activation(v_slab[:, g, 0:64], ps[:, 0:64], AF.Copy)
                nc.scalar.activation(v_slab[:, g, 65:129], ps[:, 64:128], AF.Copy)

        # ---- dense block: r/sigma as columns for all 32 token chunks
        for g in range(SCH):
            csl = slice(g * 128, (g + 1) * 128)
            pr = pss.tile([128, 1], f32, tag="small")
            nc.tensor.matmul(pr[:], r_rows[0:1, csl], ones_f1[:])
            nc.scalar.activation(ln_r_cols[:, g:g + 1], pr[:], AF.Ln)
            psg = pss.tile([128, 1], f32, tag="small")
            nc.tensor.matmul(psg[:], sig_rows[0:1, csl], ones_f1[:])
            nc.scalar.activation(sig_cols[:, g:g + 1], psg[:], AF.Copy)
            nc.vector.tensor_copy(v_slab[:, g, 64:65], sig_cols[:, g:g + 1])
            nc.vector.tensor_copy(v_slab[:, g, 129:130], sig_cols[:, g:g + 1])

        # ---- attention (head-split: h0 fully, A2A#0 fires, then h1)
        for hh in (0, 1):
            hsl = slice(hh * 64, (hh + 1) * 64)
            vof = 65 * hh
            for b in range(B):
                for tj in range(T // TT):
                    tsl = slice(b * T + tj * TT, b * T + (tj + 1) * TT)
                    po = psb.tile([65, TT], f32, tag="big")
                    n_si = 4 * tj + 4
                    # crossing (masked) chunks first so the tile tail has a
                    # short scores->exp->PV chain
                    order = list(range(4 * tj, n_si)) + list(range(0, 4 * tj))
                    for idx, si in enumerate(order):
                        g = b * (T // 128) + si
                        ssl = slice(g * 128, (g + 1) * 128)
                        ps = psb.tile([128, TT], f32, tag="big")
                        nc.tensor.matmul(ps[:], kT[hsl, ssl], qT[hsl, tsl],
                                         start=True, stop=True)
                        ex = expp.tile([128, TT], att_dt, tag="ex")
                        nc.scalar.activation(
                            ex[:], ps[:], AF.Exp,
                            scale=float(E) ** -0.5,
                            bias=ln_r_cols[:, g:g + 1],
                        )
                        if si >= 4 * tj:
                            nc.vector.tensor_mul(ex[:], ex[:],
                                                 masks[:, si - 4 * tj, :])
                        nc.tensor.matmul(po[:], v_slab[:, g, vof:vof + 65],
                                         ex[:], start=(idx == 0),
                                         stop=(idx == n_si - 1))
                    rd = expp.tile([1, TT], f32, tag="rd")
                    nc.vector.reciprocal(rd[:], po[64:65, :])
                    rb = expp.tile([64, TT], f32, tag="rb")
                    nc.gpsimd.partition_broadcast(rb[:], rd[:])
                    nc.vector.tensor_mul(attn_slab[hsl, tsl],
                                         po[0:64, :], rb[:])
                    d = b * (T // TT) + tj
                    nc.sync.dma_start(a2a_in[hh][d],
                                      attn_slab[hsl, d * TB:(d + 1) * TB])
            nc.gpsimd.collective_compute(
                "AllToAll", mybir.AluOpType.bypass,
                ins=[a2a_in[hh].opt()], outs=[a2a_out[hh].opt()],
                replica_groups=[list(range(W))],
            )

        es_a.close()

        # ========================================================= PHASE B
        phb = es.enter_context(tc.tile_pool(name="phb", bufs=1))
        str_b = es.enter_context(tc.tile_pool(name="strb", bufs=2))

        atf = phb.tile([128, FCH, TB], mm_dt, tag="atf")
        for o in range(FCH):
            nc.sync.dma_start(atf[0:64, o, :], a2a_out[0][o])
            nc.sync.dma_start(atf[64:128, o, :], a2a_out[1][o])
        xtb = phb.tile([128, FCH, TB], f32, tag="xtb")
        for o in range(FCH):
            nc.sync.dma_start(xtb[:, o, :], xtb_d[o])
        bproj = phb.tile([128, FCH], f32, tag="bproj")
        nc.sync.dma_start(bproj[:], bproj_d[:])
        b2c = phb.tile([128, FCH], f32, tag="b2c")
        nc.sync.dma_start(b2c[:], b2_d[:])
        augw1 = phb.tile([2, M4E], mm_dt, tag="augw1")
        nc.sync.dma_start(augw1[:], augw1_d[:])

        # ---- proj + residual -> x2T
        x2T = phb.tile([128, FCH, TB], f32, tag="x2T")
        for et in range(FCH):
            esl = slice(et * 128, (et + 1) * 128)
            wp = str_b.tile([128, FCH, 128], mm_dt, tag="wp")
            nc.sync.dma_start(wp[:], wproj_d[et].rearrange("p (o c) -> p o c", c=128))
            ps = psb.tile([128, TB], f32, tag="big")
            for o in range(FCH):
                nc.tensor.matmul(ps[:], wp[:, o, :], atf[:, o, :],
                                 start=(o == 0), stop=(o == FCH - 1))
            nc.vector.scalar_tensor_tensor(
                x2T[:, et, :], ps[:], bproj[:, et:et + 1], xtb[:, et, :],
                mybir.AluOpType.add, mybir.AluOpType.add)

        # ---- LN2 stats
        xh2 = phb.tile([128, FCH, TB], mybir.dt.bfloat16, tag="xh2")
        ps_sum = pss.tile([1, TB], f32, tag="small")
        ps_sq = pss.tile([1, TB], f32, tag="small")
        for o in range(FCH):
            nc.vector.tensor_copy(xh2[:, o, :], x2T[:, o, :])
            sq = str_b.tile([128, TB], mybir.dt.bfloat16, tag="sq2")
            nc.scalar.activation(sq[:], xh2[:, o, :], AF.Square)
            nc.tensor.matmul(ps_sum[:], ones_bf[:], xh2[:, o, :],
                             start=(o == 0), stop=(o == FCH - 1))
            nc.tensor.matmul(ps_sq[:], ones_bf[:], sq[:],
                             start=(o == 0), stop=(o == FCH - 1))
        mu2 = phb.tile([1, TB], f32, tag="mu2")
        nc.scalar.activation(mu2[:], ps_sum[:], AF.Copy, scale=1.0 / E)
        msq2 = phb.tile([1, TB], f32, tag="msq2")
        nc.scalar.activation(msq2[:], ps_sq[:], AF.Copy, scale=1.0 / E)
        var2 = phb.tile([1, TB], f32, tag="var2")
        nc.vector.tensor_mul(var2[:], mu2[:], mu2[:])
        nc.vector.tensor_sub(var2[:], msq2[:], var2[:])
        sig2 = phb.tile([1, TB], f32, tag="sig2")
        nc.scalar.activation(sig2[:], var2[:], AF.Sqrt, bias=eps1[:])
        r2 = phb.tile([1, TB], f32, tag="r2")
        nc.vector.reciprocal(r2[:], sig2[:])
        aug2 = phb.tile([2, TB], mm_dt, tag="aug2")
        nc.vector.tensor_copy(aug2[0:1, :], mu2[:])
        nc.gpsimd.dma_start(aug2[1:2, :], sig2[:])
        r2_b = phb.tile([128, TB], f32, tag="r2_b")
        nc.gpsimd.partition_broadcast(r2_b[:], r2[:])

        x2mm = x2T if use_f32_x else xh2

        # ---- FFN1 -> relu slab (r2 deferred to FFN2 output: r2>0)
        relu = phb.tile([128, MCH, TB], mm_dt, tag="relu")
        for st in range(FCH):
            w1t = str_b.tile([128, FCH, 512], mm_dt, tag="w1t")
            nc.sync.dma_start(w1t[:], w1_d[st].rearrange("p (o c) -> p o c", c=512))
            for mj in range(4):
                mt = st * 4 + mj
                msl = slice(mt * 128, (mt + 1) * 128)
                jsl = slice(mj * 128, (mj + 1) * 128)
                ps = psb.tile([128, TB], f32, tag="big")
                for o in range(FCH):
                    nc.tensor.matmul(ps[:], w1t[:, o, jsl], x2mm[:, o, :],
                                     start=(o == 0), stop=False)
                nc.tensor.matmul(ps[:], augw1[:, msl], aug2[:],
                                 start=False, stop=True)
                nc.scalar.activation(relu[:, mt, :], ps[:], AF.Relu)

        # ---- FFN2 + r2 + residual + b2 -> y
        for et in range(FCH):
            ps = psb.tile([128, TB], f32, tag="big")
            for qq in range(4):
                w2t = str_b.tile([128, MCH // 4, 128], mm_dt, tag="w2t")
                nc.sync.dma_start(
                    w2t[:],
                    w2_d[et, :, qq * (M4E // 4):(qq + 1) * (M4E // 4)]
                    .rearrange("p (m f) -> p m f", f=128))
                for mj in range(MCH // 4):
                    mc = qq * (MCH // 4) + mj
                    nc.tensor.matmul(ps[:], w2t[:, mj, :], relu[:, mc, :],
                                     start=(mc == 0), stop=(mc == MCH - 1))
            u = str_b.tile([128, TB], f32, tag="u")
            nc.vector.tensor_mul(u[:], ps[:], r2_b[:])
            yt = str_b.tile([128, TB], f32, tag="yt")
            nc.vector.scalar_tensor_tensor(
                yt[:], u[:], b2c[:, et:et + 1], x2T[:, et, :],
                mybir.AluOpType.add, mybir.AluOpType.add)
            nc.sync.dma_start(y_d[et], yt[:])


    nc.finalize()
    return nc


def _get_nc():
    key = (MM_DT_S, ATT_DT_S)
    if key in _CACHE:
        return _CACHE[key]
    from concourse import bacc
    import concourse.mybir as mybir

    f32 = mybir.dt.float32
    mm_dt = f32 if MM_DT_S == "f32" else mybir.dt.bfloat16
    att_dt = f32 if ATT_DT_S == "f32" else mybir.dt.bfloat16
    nc = bacc.Bacc("TRN2", target_bir_lowering=False, debug=False,
                   num_devices=W)
    _build(nc, mm_dt, att_dt, f32)
    _CACHE[key] = nc
    return nc


def _prep_inputs(x, wq, wk, wv, w_proj, b_proj, w1, b1, w2, b2, g1, be1, g2, be2):
    """Host-side sharding: returns in_maps (list of 8 dicts)."""
    import ml_dtypes

    bf16 = ml_dtypes.bfloat16
    mm_np = np.float32 if MM_DT_S == "f32" else bf16
    att_np = np.float32 if ATT_DT_S == "f32" else bf16

    xf = np.ascontiguousarray(x.reshape(BT, E).T)          # [E, BT]
    xT = xf.reshape(FCH, 128, BT)
    xh = xT.astype(bf16)

    # causal mask tiles for the 4 diagonal-crossing offsets
    mask = np.zeros((4, 128, TT), dtype=att_np)
    uu = np.arange(TT)[None, :]
    pp = np.arange(128)[:, None]
    for k in range(4):
        mask[k] = (pp <= uu - 128 * k).astype(att_np)

    # [et][p][(o, c128)]: wproj_l[et, p, o*128+c] = w_proj[o*128+p, et*128+c]
    wpr = w_proj.reshape(FCH, 128, FCH, 128)                # [o, p, et, c]
    wproj_l = np.ascontiguousarray(wpr.transpose(2, 1, 0, 3).reshape(FCH, 128, E)).astype(mm_np)
    bproj_l = np.ascontiguousarray(b_proj.reshape(FCH, 128).T)  # [128, FCH]

    w1s = (g2[:, None] * w1)                                # [E, 4E]
    # [s][p][(o, c512)]: w1_l[s, p, o*512+c] = w1s[o*128+p, s*512+c]
    w1r = w1s.reshape(FCH, 128, FCH, 512)                   # [o, p, s, c]
    w1_l = np.ascontiguousarray(w1r.transpose(2, 1, 0, 3).reshape(FCH, 128, M4E)).astype(mm_np)
    aug1 = np.stack([-w1s.sum(axis=0), be2 @ w1 + b1]).astype(mm_np)

    # w2 host layout: [et, p, (mc, f)] with w2_l[et, p, mc*128+f] = w2[mc*128+p, et*128+f]
    w2r = w2.reshape(MCH, 128, FCH, 128)                    # [mc, p, et, f]
    w2_l = np.ascontiguousarray(w2r.transpose(2, 1, 0, 3).reshape(FCH, 128, M4E)).astype(mm_np)
    b2_l = np.ascontiguousarray(b2.reshape(FCH, 128).T)

    in_maps = []
    for c in range(W):
        hsl = slice(HPC * c, HPC * (c + 1))
        wq_c = wq[hsl].transpose(1, 0, 2).reshape(E, 128)
        wk_c = wk[hsl].transpose(1, 0, 2).reshape(E, 128)
        wv_c = wv[hsl].transpose(1, 0, 2).reshape(E, 128)
        wqkv = np.concatenate([g1[:, None] * wq_c,
                               g1[:, None] * wk_c,
                               g1[:, None] * wv_c], axis=1)  # [E, 384]
        augw = np.stack([-wqkv.sum(axis=0),
                         np.concatenate([be1 @ wq_c, be1 @ wk_c, be1 @ wv_c])]
                        ).astype(mm_np)
        m = {
            "xh": xh,
            "xtb": np.ascontiguousarray(xT[:, :, TB * c:TB * (c + 1)]),
            "wqkv": np.ascontiguousarray(wqkv.reshape(FCH, 128, 384)).astype(mm_np),
            "augw": augw,
            "wproj": wproj_l,
            "bproj": np.ascontiguousarray(bproj_l),
            "w1": w1_l,
            "augw1": aug1,
            "w2": w2_l,
            "b2c": np.ascontiguousarray(b2_l),
            "mask": mask,
        }
        if MM_DT_S == "f32":
            m["xT"] = xT
        in_maps.append(m)
    return in_maps


def kernel(**inputs):
    from concourse.bass_utils import run_bass_kernel_spmd

    nc = _get_nc()
    in_maps = _prep_inputs(**{k: np.asarray(v) for k, v in inputs.items()})
    res = run_bass_kernel_spmd(nc, in_maps, list(range(W)))
    # gather: core c produced y = [FCH, 128, TB] = yT block for tokens [TB*c, TB*(c+1))
    out_T = np.concatenate([res.results[c]["y"].reshape(E, TB)
                            for c in range(W)], axis=1)      # [E, BT]
    return np.ascontiguousarray(out_T.T).reshape(B, T, E).astype(np.float32)


# revision 16
# speedup vs baseline: 1.2364x; 1.0346x over previous
"""Trainium2 Bass kernel for a dense pre-LN transformer block.

Reference computation (B=2, T=2048, E=1024, H=16, HS=64):
    h  = LN(x; g1, be1)
    q, k, v = per-head projections of h        (wq/wk/wv: [H, E, HS])
    att = causal softmax(q k^T / sqrt(E)) v    (per head)
    x2 = x + concat(att) @ w_proj + b_proj
    y  = x2 + relu(LN(x2; g2, be2) @ w1 + b1) @ w2 + b2

Distribution over 8 NeuronCores:
  - QKV + attention: tensor-parallel over heads (2 heads/core).
  - proj + FFN: data-parallel over token rows (512 tokens/core).
  - One AllToAll (attn output, feature-major) glues the two.

Device-side layout is feature-major ("transposed"): activations are
[feature, token]; the host pre-transposes x and pre-arranges weights so
the kernel never transposes activations on device.

LayerNorm is folded into the following matmul:
    LN(x) @ W = r_t * (x @ (g*W) - mu_t * colsum(g*W) + sigma_t * (be@W))
applied via two rank-1 "augmentation" rows in the contraction, where
mu/sigma/r are per-token stats computed with ones-matmuls (bf16, exact
to ~1e-4). Softmax runs without max-subtraction (logits are bounded by
design: scale 1/sqrt(E)=1/32); the per-token r for V is folded into the
exp() bias as ln(r_s), and the softmax denominator is obtained from a
sigma-column appended to V in the PV matmul.
"""

import sys
from contextlib import ExitStack
import numpy as np

sys.path.insert(0, "/opt/trn_rl_repo")

# ---------------------------------------------------------------- constants
B, T, E, H = 2, 2048, 1024, 16
HS = E // H          # 64
W = 8                # cores
BT = B * T           # 4096 tokens total
TB = BT // W         # 512 tokens per core (row-DP block)
HPC = H // W         # 2 heads per core
FCH = E // 128       # 8 feature chunks
M4E = 4 * E          # 4096 ffn hidden
MCH = M4E // 128     # 32 hidden chunks
TT = 512             # token tile (matmul moving dim)
NTT = BT // TT       # 8 token tiles
SCH = BT // 128      # 32 token chunks of 128 (for v / s-chunks)
EPS = 1e-5

# dtype knobs: "f32" or "bf16"
import os as _os
MM_DT_S = _os.environ.get("KMM_DT", "bf16")    # linear-layer matmuls
ATT_DT_S = _os.environ.get("KATT_DT", "bf16")  # attention matmuls

_CACHE = {}


def _build(nc, mm_dt, att_dt, f32):
    import concourse.bass as bass
    from concourse.tile import TileContext
    import concourse.mybir as mybir

    AF = mybir.ActivationFunctionType
    dp = nc.declare_dram_parameter

    use_f32_x = mm_dt == f32  # stream fp32 x for linear matmuls

    # ------------------------------------------------- DRAM parameters
    if use_f32_x:
        xT_d = dp("xT", [FCH, 128, BT], f32, isOutput=False)
    xh_d = dp("xh", [FCH, 128, BT], mybir.dt.bfloat16, isOutput=False)
    xtb_d = dp("xtb", [FCH, 128, TB], f32, isOutput=False)
    wqkv_d = dp("wqkv", [FCH, 128, 3 * 128], mm_dt, isOutput=False)
    augw_d = dp("augw", [2, 3 * 128], mm_dt, isOutput=False)
    wproj_d = dp("wproj", [FCH, 128, E], mm_dt, isOutput=False)
    bproj_d = dp("bproj", [128, FCH], f32, isOutput=False)
    w1_d = dp("w1", [FCH, 128, M4E], mm_dt, isOutput=False)  # [s][p][(o,c512)] layout
    augw1_d = dp("augw1", [2, M4E], mm_dt, isOutput=False)
    w2_d = dp("w2", [FCH, 128, M4E], mm_dt, isOutput=False)  # [et,p,(mc,f)] host layout
    b2_d = dp("b2c", [128, FCH], f32, isOutput=False)
    mask_d = dp("mask", [4, 128, TT], att_dt, isOutput=False)
    y_d = dp("y", [FCH, 128, TB], f32, isOutput=True)


    es = ExitStack()
    with TileContext(nc) as tc, es:
        # ------------------------------------------------- pools
        glob = es.enter_context(tc.tile_pool(name="glob", bufs=1))
        dramp = es.enter_context(tc.tile_pool(name="dramp", bufs=1, space="DRAM"))
        a2a_in = [dramp.tile([W, 64, TB], mm_dt, tag=f"a2a_in{h}", name=f"a2a_in{h}") for h in (0, 1)]
        a2a_out = [dramp.tile([W, 64, TB], mm_dt, tag=f"a2a_out{h}", name=f"a2a_out{h}") for h in (0, 1)]
        psb = es.enter_context(tc.tile_pool(name="psb", bufs=6, space="PSUM"))
        pss = es.enter_context(tc.tile_pool(name="pss", bufs=2, space="PSUM"))

        ones_bf = glob.tile([128, 1], mybir.dt.bfloat16, tag="ones_bf")
        nc.vector.memset(ones_bf[:], 1.0)
        ones_f1 = glob.tile([1, 1], f32, tag="ones_f1")
        nc.vector.memset(ones_f1[:], 1.0)

        eps1 = glob.tile([1, 1], f32, tag="eps1")
        nc.vector.memset(eps1[:], EPS)
        ln_r_cols = glob.tile([128, SCH], f32, tag="ln_r_cols")
        sig_cols = glob.tile([128, SCH], f32, tag="sig_cols")
        attn_slab = glob.tile([128, BT], mm_dt, tag="attn_slab")

        # ========================================================= PHASE A
        es_a = es.enter_context(ExitStack())
        pha = es_a.enter_context(tc.tile_pool(name="pha", bufs=1))
        str_a = es_a.enter_context(tc.tile_pool(name="stra", bufs=2))
        expp = es_a.enter_context(tc.tile_pool(name="expp", bufs=18))

        wqkv = pha.tile([128, FCH, 3 * 128], mm_dt, tag="wqkv")
        for o in range(FCH):
            nc.sync.dma_start(wqkv[:, o, :], wqkv_d[o])
        augw = pha.tile([2, 3 * 128], mm_dt, tag="augw")
        nc.sync.dma_start(augw[:], augw_d[:])
        masks = pha.tile([128, 4, TT], att_dt, tag="masks")
        for kk in range(4):
            nc.sync.dma_start(masks[:, kk, :], mask_d[kk])

        sig_rows = pha.tile([1, BT], f32, tag="sig_rows")
        r_rows = pha.tile([1, BT], f32, tag="r_rows")
        qT = pha.tile([128, BT], att_dt, tag="qT")
        kT = pha.tile([128, BT], att_dt, tag="kT")
        v_slab = pha.tile([128, SCH, 130], att_dt, tag="v_slab")

        # ---- per token-tile: stats + QKV
        for tj in range(NTT):
            tsl = slice(tj * TT, (tj + 1) * TT)
            if use_f32_x:
                xt = str_a.tile([128, FCH, TT], f32, tag="xt")
                for o in range(FCH):
                    nc.sync.dma_start(xt[:, o, :], xT_d[o, :, tsl])
            xh = str_a.tile([128, FCH, TT], mybir.dt.bfloat16, tag="xh")
            for o in range(FCH):
                nc.sync.dma_start(xh[:, o, :], xh_d[o, :, tsl])

            # token stats: sum(x), sum(x^2) over features (bf16 ones-matmuls)
            ps_sum = pss.tile([1, TT], f32, tag="small")
            ps_sq = pss.tile([1, TT], f32, tag="small")
            for o in range(FCH):
                sq = str_a.tile([128, TT], mybir.dt.bfloat16, tag="sq")
                nc.scalar.activation(sq[:], xh[:, o, :], AF.Square)
                nc.tensor.matmul(ps_sum[:], ones_bf[:], xh[:, o, :],
                                 start=(o == 0), stop=(o == FCH - 1))
                nc.tensor.matmul(ps_sq[:], ones_bf[:], sq[:],
                                 start=(o == 0), stop=(o == FCH - 1))

            mu_row = str_a.tile([1, TT], f32, tag="mu_row")
            nc.scalar.activation(mu_row[:], ps_sum[:], AF.Copy, scale=1.0 / E)
            msq_row = str_a.tile([1, TT], f32, tag="msq_row")
            nc.scalar.activation(msq_row[:], ps_sq[:], AF.Copy, scale=1.0 / E)
            var_row = str_a.tile([1, TT], f32, tag="var_row")
            nc.vector.tensor_mul(var_row[:], mu_row[:], mu_row[:])
            nc.vector.tensor_sub(var_row[:], msq_row[:], var_row[:])
            sig_row = sig_rows[0:1, tsl]
            nc.scalar.activation(sig_row, var_row[:], AF.Sqrt, bias=eps1[:])
            r_row = r_rows[0:1, tsl]
            nc.vector.reciprocal(r_row, sig_row)

            aug_rhs = str_a.tile([2, TT], mm_dt, tag="aug_rhs")
            nc.vector.tensor_copy(aug_rhs[0:1, :], mu_row[:])
            nc.gpsimd.dma_start(aug_rhs[1:2, :], sig_row)

            r_b = str_a.tile([128, TT], f32, tag="r_b")
            nc.gpsimd.partition_broadcast(r_b[:], r_row)

            x_mm = xt if use_f32_x else xh

            # q, k: out = [feat, tok]
            for fg, slab in ((0, qT), (1, kT)):
                fsl = slice(fg * 128, (fg + 1) * 128)
                ps = psb.tile([128, TT], f32, tag="big")
                for o in range(FCH):
                    nc.tensor.matmul(ps[:], wqkv[:, o, fsl], x_mm[:, o, :],
                                     start=(o == 0), stop=False)
                nc.tensor.matmul(ps[:], augw[:, fsl], aug_rhs[:],
                                 start=False, stop=True)
                nc.vector.tensor_mul(slab[:, tsl], ps[:], r_b[:])

            # v: out = [tok, feat] (row-major, for PV lhsT)
            for j in range(4):
                g = tj * 4 + j
                csl = slice(j * 128, (j + 1) * 128)
                ps = psb.tile([128, 128], f32, tag="big")
                for o in range(FCH):
                    nc.tensor.matmul(ps[:], x_mm[:, o, csl],
                                     wqkv[:, o, 256:384],
                                     start=(o == 0), stop=False)
                nc.tensor.matmul(ps[:], aug_rhs[:, csl], augw[:, 256:384],
                                 start=False, stop=True)
                nc.scalar.activation(v_slab[:, g, 0:64], ps[:, 0:64], AF.Copy)
                nc.scalar.activation(v_slab[:, g, 65:129], ps[:, 64:128], AF.Copy)

        # ---- dense block: r/sigma as columns for all 32 token chunks
        for g in range(SCH):
            csl = slice(g * 128, (g + 1) * 128)
            pr = pss.tile([128, 1], f32, tag="small")
            nc.tensor.matmul(pr[:], r_rows[0:1, csl], ones_f1[:])
            nc.scalar.activation(ln_r_cols[:, g:g + 1], pr[:], AF.Ln)
            psg = pss.tile([128, 1], f32, tag="small")
            nc.tensor.matmul(psg[:], sig_rows[0:1, csl], ones_f1[:])
            nc.scalar.activation(sig_cols[:, g:g + 1], psg[:], AF.Copy)
            nc.vector.tensor_copy(v_slab[:, g, 64:65], sig_cols[:, g:g + 1])
            nc.vector.tensor_copy(v_slab[:, g, 129:130], sig_cols[:, g:g + 1])

        # ---- attention (head-split: h0 fully, A2A#0 fires, then h1)
        for hh in (0, 1):
            hsl = slice(hh * 64, (hh + 1) * 64)
            vof = 65 * hh
            for b in range(B):
                for tj in range(T // TT):
                    tsl = slice(b * T + tj * TT, b * T + (tj + 1) * TT)
                    po = psb.tile([65, TT], f32, tag="big")
                    n_si = 4 * tj + 4
                    # all scores issued first (exp/mask run on ACT/DVE in
                    # parallel), then the PV accumulation chain
                    exs = []
                    for si in range(n_si):
                        g = b * (T // 128) + si
                        ssl = slice(g * 128, (g + 1) * 128)
                        ps = psb.tile([128, TT], f32, tag="big")
                        nc.tensor.matmul(ps[:], kT[hsl, ssl], qT[hsl, tsl],
                                         start=True, stop=True)
                        ex = expp.tile([128, TT], att_dt, tag="ex")
                        nc.scalar.activation(
                            ex[:], ps[:], AF.Exp,
                            scale=float(E) ** -0.5,
                            bias=ln_r_cols[:, g:g + 1],
                        )
                        if si >= 4 * tj:
                            nc.vector.tensor_mul(ex[:], ex[:],
                                                 masks[:, si - 4 * tj, :])
                        exs.append((g, ex))
                    for idx, (g, ex) in enumerate(exs):
                        nc.tensor.matmul(po[:], v_slab[:, g, vof:vof + 65],
                                         ex[:], start=(idx == 0),
                                         stop=(idx == n_si - 1))
                    rd = expp.tile([1, TT], f32, tag="rd")
                    nc.vector.reciprocal(rd[:], po[64:65, :])
                    rb = expp.tile([64, TT], f32, tag="rb")
                    nc.gpsimd.partition_broadcast(rb[:], rd[:])
                    nc.vector.tensor_mul(attn_slab[hsl, tsl],
                                         po[0:64, :], rb[:])
                    d = b * (T // TT) + tj
                    nc.sync.dma_start(a2a_in[hh][d],
                                      attn_slab[hsl, d * TB:(d + 1) * TB])
            nc.gpsimd.collective_compute(
                "AllToAll", mybir.AluOpType.bypass,
                ins=[a2a_in[hh].opt()], outs=[a2a_out[hh].opt()],
                replica_groups=[list(range(W))],
            )

        es_a.close()

        # ========================================================= PHASE B
        phb = es.enter_context(tc.tile_pool(name="phb", bufs=1))
        str_b = es.enter_context(tc.tile_pool(name="strb", bufs=2))

        atf = phb.tile([128, FCH, TB], mm_dt, tag="atf")
        for o in range(FCH):
            nc.sync.dma_start(atf[0:64, o, :], a2a_out[0][o])
            nc.sync.dma_start(atf[64:128, o, :], a2a_out[1][o])
        xtb = phb.tile([128, FCH, TB], f32, tag="xtb")
        for o in range(FCH):
            nc.sync.dma_start(xtb[:, o, :], xtb_d[o])
        bproj = phb.tile([128, FCH], f32, tag="bproj")
        nc.sync.dma_start(bproj[:], bproj_d[:])
        b2c = phb.tile([128, FCH], f32, tag="b2c")
        nc.sync.dma_start(b2c[:], b2_d[:])
        augw1 = phb.tile([2, M4E], mm_dt, tag="augw1")
        nc.sync.dma_start(augw1[:], augw1_d[:])

        # ---- proj + residual -> x2T
        x2T = phb.tile([128, FCH, TB], f32, tag="x2T")
        for et in range(FCH):
            esl = slice(et * 128, (et + 1) * 128)
            wp = str_b.tile([128, FCH, 128], mm_dt, tag="wp")
            nc.sync.dma_start(wp[:], wproj_d[et].rearrange("p (o c) -> p o c", c=128))
            ps = psb.tile([128, TB], f32, tag="big")
            for o in range(FCH):
                nc.tensor.matmul(ps[:], wp[:, o, :], atf[:, o, :],
                                 start=(o == 0), stop=(o == FCH - 1))
            nc.vector.scalar_tensor_tensor(
                x2T[:, et, :], ps[:], bproj[:, et:et + 1], xtb[:, et, :],
                mybir.AluOpType.add, mybir.AluOpType.add)

        # ---- LN2 stats
        xh2 = phb.tile([128, FCH, TB], mybir.dt.bfloat16, tag="xh2")
        ps_sum = pss.tile([1, TB], f32, tag="small")
        ps_sq = pss.tile([1, TB], f32, tag="small")
        for o in range(FCH):
            nc.vector.tensor_copy(xh2[:, o, :], x2T[:, o, :])
            sq = str_b.tile([128, TB], mybir.dt.bfloat16, tag="sq2")
            nc.scalar.activation(sq[:], xh2[:, o, :], AF.Square)
            nc.tensor.matmul(ps_sum[:], ones_bf[:], xh2[:, o, :],
                             start=(o == 0), stop=(o == FCH - 1))
            nc.tensor.matmul(ps_sq[:], ones_bf[:], sq[:],
                             start=(o == 0), stop=(o == FCH - 1))
        mu2 = phb.tile([1, TB], f32, tag="mu2")
        nc.scalar.activation(mu2[:], ps_sum[:], AF.Copy, scale=1.0 / E)
        msq2 = phb.tile([1, TB], f32, tag="msq2")
        nc.scalar.activation(msq2[:], ps_sq[:], AF.Copy, scale=1.0 / E)
        var2 = phb.tile([1, TB], f32, tag="var2")
        nc.vector.tensor_mul(var2[:], mu2[:], mu2[:])
        nc.vector.tensor_sub(var2[:], msq2[:], var2[:])
        sig2 = phb.tile([1, TB], f32, tag="sig2")
        nc.scalar.activation(sig2[:], var2[:], AF.Sqrt, bias=eps1[:])
        r2 = phb.tile([1, TB], f32, tag="r2")
        nc.vector.reciprocal(r2[:], sig2[:])
        aug2 = phb.tile([2, TB], mm_dt, tag="aug2")
        nc.vector.tensor_copy(aug2[0:1, :], mu2[:])
        nc.gpsimd.dma_start(aug2[1:2, :], sig2[:])
        r2_b = phb.tile([128, TB], f32, tag="r2_b")
        nc.gpsimd.partition_broadcast(r2_b[:], r2[:])

        x2mm = x2T if use_f32_x else xh2

        # ---- FFN1 -> relu slab (r2 deferred to FFN2 output: r2>0)
        relu = phb.tile([128, MCH, TB], mm_dt, tag="relu")
        for st in range(FCH):
            w1t = str_b.tile([128, FCH, 512], mm_dt, tag="w1t")
            nc.sync.dma_start(w1t[:], w1_d[st].rearrange("p (o c) -> p o c", c=512))
            for mj in range(4):
                mt = st * 4 + mj
                msl = slice(mt * 128, (mt + 1) * 128)
                jsl = slice(mj * 128, (mj + 1) * 128)
                ps = psb.tile([128, TB], f32, tag="big")
                for o in range(FCH):
                    nc.tensor.matmul(ps[:], w1t[:, o, jsl], x2mm[:, o, :],
                                     start=(o == 0), stop=False)
                nc.tensor.matmul(ps[:], augw1[:, msl], aug2[:],
                                 start=False, stop=True)
                nc.scalar.activation(relu[:, mt, :], ps[:], AF.Relu)

        # ---- FFN2 + r2 + residual + b2 -> y
        for et in range(FCH):
            ps = psb.tile([128, TB], f32, tag="big")
            for qq in range(4):
                w2t = str_b.tile([128, MCH // 4, 128], mm_dt, tag="w2t")
                nc.sync.dma_start(
                    w2t[:],
                    w2_d[et, :, qq * (M4E // 4):(qq + 1) * (M4E // 4)]
                    .rearrange("p (m f) -> p m f", f=128))
                for mj in range(MCH // 4):
                    mc = qq * (MCH // 4) + mj
                    nc.tensor.matmul(ps[:], w2t[:, mj, :], relu[:, mc, :],
                                     start=(mc == 0), stop=(mc == MCH - 1))
            u = str_b.tile([128, TB], f32, tag="u")
            nc.vector.tensor_mul(u[:], ps[:], r2_b[:])
            yt = str_b.tile([128, TB], f32, tag="yt")
            nc.vector.scalar_tensor_tensor(
                yt[:], u[:], b2c[:, et:et + 1], x2T[:, et, :],
                mybir.AluOpType.add, mybir.AluOpType.add)
            nc.sync.dma_start(y_d[et], yt[:])


    nc.finalize()
    return nc


def _get_nc():
    key = (MM_DT_S, ATT_DT_S)
    if key in _CACHE:
        return _CACHE[key]
    from concourse import bacc
    import concourse.mybir as mybir

    f32 = mybir.dt.float32
    mm_dt = f32 if MM_DT_S == "f32" else mybir.dt.bfloat16
    att_dt = f32 if ATT_DT_S == "f32" else mybir.dt.bfloat16
    nc = bacc.Bacc("TRN2", target_bir_lowering=False, debug=False,
                   num_devices=W)
    _build(nc, mm_dt, att_dt, f32)
    _CACHE[key] = nc
    return nc


def _prep_inputs(x, wq, wk, wv, w_proj, b_proj, w1, b1, w2, b2, g1, be1, g2, be2):
    """Host-side sharding: returns in_maps (list of 8 dicts)."""
    import ml_dtypes

    bf16 = ml_dtypes.bfloat16
    mm_np = np.float32 if MM_DT_S == "f32" else bf16
    att_np = np.float32 if ATT_DT_S == "f32" else bf16

    xf = np.ascontiguousarray(x.reshape(BT, E).T)          # [E, BT]
    xT = xf.reshape(FCH, 128, BT)
    xh = xT.astype(bf16)

    # causal mask tiles for the 4 diagonal-crossing offsets
    mask = np.zeros((4, 128, TT), dtype=att_np)
    uu = np.arange(TT)[None, :]
    pp = np.arange(128)[:, None]
    for k in range(4):
        mask[k] = (pp <= uu - 128 * k).astype(att_np)

    # [et][p][(o, c128)]: wproj_l[et, p, o*128+c] = w_proj[o*128+p, et*128+c]
    wpr = w_proj.reshape(FCH, 128, FCH, 128)                # [o, p, et, c]
    wproj_l = np.ascontiguousarray(wpr.transpose(2, 1, 0, 3).reshape(FCH, 128, E)).astype(mm_np)
    bproj_l = np.ascontiguousarray(b_proj.reshape(FCH, 128).T)  # [128, FCH]

    w1s = (g2[:, None] * w1)                                # [E, 4E]
    # [s][p][(o, c512)]: w1_l[s, p, o*512+c] = w1s[o*128+p, s*512+c]
    w1r = w1s.reshape(FCH, 128, FCH, 512)                   # [o, p, s, c]
    w1_l = np.ascontiguousarray(w1r.transpose(2, 1, 0, 3).reshape(FCH, 128, M4E)).astype(mm_np)
    aug1 = np.stack([-w1s.sum(axis=0), be2 @ w1 + b1]).astype(mm_np)

    # w2 host layout: [et, p, (mc, f)] with w2_l[et, p, mc*128+f] = w2[mc*128+p, et*128+f]
    w2r = w2.reshape(MCH, 128, FCH, 128)                    # [mc, p, et, f]
    w2_l = np.ascontiguousarray(w2r.transpose(2, 1, 0, 3).reshape(FCH, 128, M4E)).astype(mm_np)
    b2_l = np.ascontiguousarray(b2.reshape(FCH, 128).T)

    in_maps = []
    for c in range(W):
        hsl = slice(HPC * c, HPC * (c + 1))
        wq_c = wq[hsl].transpose(1, 0, 2).reshape(E, 128)
        wk_c = wk[hsl].transpose(1, 0, 2).reshape(E, 128)
        wv_c = wv[hsl].transpose(1, 0, 2).reshape(E, 128)
        wqkv = np.concatenate([g1[:, None] * wq_c,
                               g1[:, None] * wk_c,
                               g1[:, None] * wv_c], axis=1)  # [E, 384]
        augw = np.stack([-wqkv.sum(axis=0),
                         np.concatenate([be1 @ wq_c, be1 @ wk_c, be1 @ wv_c])]
                        ).astype(mm_np)
        m = {
            "xh": xh,
            "xtb": np.ascontiguousarray(xT[:, :, TB * c:TB * (c + 1)]),
            "wqkv": np.ascontiguousarray(wqkv.reshape(FCH, 128, 384)).astype(mm_np),
            "augw": augw,
            "wproj": wproj_l,
            "bproj": np.ascontiguousarray(bproj_l),
            "w1": w1_l,
            "augw1": aug1,
            "w2": w2_l,
            "b2c": np.ascontiguousarray(b2_l),
            "mask": mask,
        }
        if MM_DT_S == "f32":
            m["xT"] = xT
        in_maps.append(m)
    return in_maps


def kernel(**inputs):
    from concourse.bass_utils import run_bass_kernel_spmd

    nc = _get_nc()
    in_maps = _prep_inputs(**{k: np.asarray(v) for k, v in inputs.items()})
    res = run_bass_kernel_spmd(nc, in_maps, list(range(W)))
    # gather: core c produced y = [FCH, 128, TB] = yT block for tokens [TB*c, TB*(c+1))
    out_T = np.concatenate([res.results[c]["y"].reshape(E, TB)
                            for c in range(W)], axis=1)      # [E, BT]
    return np.ascontiguousarray(out_T.T).reshape(B, T, E).astype(np.float32)
